# revision 27
# baseline (speedup 1.0000x reference)
"""DeltaNetBlock Trainium2 kernel (v4).

Sharding: 8 cores = 4 batches x 2 output-column halves. Each core computes
the full middle (proj -> conv -> silu -> l2norm -> chunked delta scan) for
its batch and the output projection for its half of the output columns.

Structure:
- Projection in 3 groups (8m / 5m / 3m PSUM banks); conv tiles interleave
  with the later groups so silu/l2 finish right after the projection.
- beta = sigmoid(x @ beta_w + b) computed on the host (input prep).
- l2 normalization folded into matmul operands per chunk on the Pool
  engine: Ks2 = rk^2*k', Kkq = rk*rq*k', Qn = rq*q', Kn = rk*k'.
- Per 128-chunk the delta recurrence is affine in S^T:
    S_{c+1}^T = S_c^T - H_c S_c^T + J_c,  O_c^T = S Qn_c - (P F S^T)^T + (P R)^T
  with R = (I+A) beta V^T, FN = -(I+A) beta Kn^T, J = Kn R, HtN = -H^T,
  GtN = -(P F)^T, PRT = (P R)^T all S-independent. J and PRT never leave
  PSUM: the scan's S/O matmuls continue their accumulation groups in place.
- rmsnorm via ones-matmul column sums; rsqrt folded into the out-proj
  result copies (per-partition scale). out_b added on host.
"""
import sys
sys.path.insert(0, '/opt/trn_rl_repo')
import numpy as np

B, L, D = 4, 2048, 128
NCHUNK = L // 128
NOUT = L // 2
HORNER = 1
_DEBUG = False


def _build_program(eps_rms: float):
    from concourse import bacc, mybir, tile

    F32 = mybir.dt.float32
    BF16 = mybir.dt.bfloat16
    ACT = mybir.ActivationFunctionType
    from concourse.alu_op_type import AluOpType

    nc = bacc.Bacc("TRN2", target_bir_lowering=False, debug=False)

    xh = nc.dram_tensor("xh", [L, L], BF16, kind="ExternalInput")
    wt = nc.dram_tensor("wt", [L, 386], BF16, kind="ExternalInput")
    bias_row_d = nc.dram_tensor("bias_row", [1, 386], BF16, kind="ExternalInput")
    conv_w = nc.dram_tensor("conv_w", [128, 1152], BF16, kind="ExternalInput")
    conv_b = nc.dram_tensor("conv_b", [128, 3], F32, kind="ExternalInput")
    ident_d = nc.dram_tensor("ident", [128, 128], BF16, kind="ExternalInput")
    mask_su_d = nc.dram_tensor("mask_su", [128, 128], BF16, kind="ExternalInput")
    mask_ui_d = nc.dram_tensor("mask_ui", [128, 128], BF16, kind="ExternalInput")
    outwt_d = nc.dram_tensor("outwt", [128, NOUT], BF16, kind="ExternalInput")
    bpos_d = nc.dram_tensor("bpos", [128, 16], F32, kind="ExternalInput")
    bneg_d = nc.dram_tensor("bneg", [128, 16], F32, kind="ExternalInput")
    out_sh = nc.dram_tensor("out_sh", [L, NOUT], F32, kind="ExternalOutput")
    if _DEBUG:
        dbg_y = nc.dram_tensor("dbg_y", [128, 6150], BF16, kind="ExternalOutput")
        dbg_kqv = nc.dram_tensor("dbg_kqv", [128, 3 * 2048], BF16, kind="ExternalOutput")
        dbg_rkrq = nc.dram_tensor("dbg_rkrq", [128, 2], F32, kind="ExternalOutput")
        dbg_st = nc.dram_tensor("dbg_st", [128, 512], BF16, kind="ExternalOutput")
        dbg_spt = nc.dram_tensor("dbg_spt", [128, 128], BF16, kind="ExternalOutput")
        dbg_kk = nc.dram_tensor("dbg_kk", [128, 512], BF16, kind="ExternalOutput")
        dbg_rf = nc.dram_tensor("dbg_rf", [128, 256], BF16, kind="ExternalOutput")

    with tile.TileContext(nc) as tc:
        with tc.tile_pool(name="const", bufs=1) as cpool, \
             tc.tile_pool(name="big", bufs=1) as bigpool:

            ones_r = cpool.tile([1, 128], BF16)
            nc.vector.memset(ones_r[:], 1.0)
            ones_c = cpool.tile([128, 1], BF16)
            nc.vector.memset(ones_c[:], 1.0)
            eps_c = cpool.tile([128, 1], F32)
            nc.vector.memset(eps_c[:], float(eps_rms))

            bias_row = cpool.tile([1, 386], BF16)
            convw_t = cpool.tile([128, 1152], BF16)
            convb_t = cpool.tile([128, 3], F32)
            ident = cpool.tile([128, 128], BF16)
            mask_su = cpool.tile([128, 128], BF16)
            mask_ui = cpool.tile([128, 128], BF16)
            outwt = cpool.tile([128, NOUT], BF16)
            beta_pos = cpool.tile([128, 16], F32)
            beta_neg = cpool.tile([128, 16], F32)
            rk = cpool.tile([128, 1], F32)
            rq = cpool.tile([128, 1], F32)
            rk2 = cpool.tile([128, 1], F32)
            rkq = cpool.tile([128, 1], F32)

            Ybig = bigpool.tile([128, 6150], BF16, name="ybig")
            Y = [Ybig[:, s * 2050:(s + 1) * 2050] for s in range(3)]
            kqv = [bigpool.tile([128, 2048], BF16, tag=f"c{s}", name=f"c{s}")
                   for s in range(3)]

            nc.sync.dma_start(bias_row[:], bias_row_d[:])
            for s in range(3):
                nc.vector.memset(Y[s][:, 0:1], 0.0)
                nc.vector.memset(Y[s][:, 2049:2050], 0.0)

            ssq_acc = {s: cpool.tile([128, 4], F32, name=f"acc{s}")
                       for s in (0, 1)}
            sq_scr = {s: bigpool.tile([128, 512], BF16, tag=f"scr{s}",
                                      name=f"scr{s}") for s in (0, 1)}

            def conv_emit(nb, cvps):
                for s in (2, 0, 1):
                    ps = cvps.tile([128, 512], F32, tag="cv", name="cv")
                    for t in range(3):
                        nc.tensor.matmul(
                            ps[:],
                            convw_t[:, (3 * s + t) * 128:(3 * s + t + 1) * 128],
                            Y[s][:, nb * 512 + t:nb * 512 + t + 512],
                            start=(t == 0), stop=(t == 2))
                    kt = kqv[s][:, nb * 512:(nb + 1) * 512]
                    nc.scalar.activation(kt, ps[:], ACT.Silu,
                                         bias=convb_t[:, s:s + 1], scale=1.0)
                    if s in (0, 1):
                        nc.scalar.activation(sq_scr[s][:], kt, ACT.Square,
                                             accum_out=ssq_acc[s][:, nb:nb + 1])

            def proj_group(jlist, xsrc, pjps):
                pj = [pjps.tile([128, 386], F32, tag=f"pj{i}", name=f"pj{i}")
                      for i in range(len(jlist))]
                for k in range(16):
                    xt = xsrc(k)
                    for i, j in enumerate(jlist):
                        nc.tensor.matmul(
                            pj[i][:], xt[:, (j % 8) * 128:(j % 8 + 1) * 128],
                            wt_tiles[k][:], start=(k == 0), stop=False)
                for i, j in enumerate(jlist):
                    nc.tensor.matmul(pj[i][:], ones_r[:], bias_row[:],
                                     start=False, stop=True)
                    nc.scalar.activation(
                        Ybig[:].rearrange("p (s r) -> p s r", s=3)[
                            :, :, 1 + 128 * j:129 + 128 * j],
                        pj[i][:, 0:384].rearrange("p (s c) -> p s c", s=3),
                        ACT.Copy)

            with tc.tile_pool(name="wtp", bufs=1) as wtpool:
                wt_tiles = [wtpool.tile([128, 386], BF16, tag=f"wt{k}",
                                        name=f"wt{k}") for k in range(16)]

                # group 0: p=0, all 8 j-blocks (8 PSUM banks), DMA interleaved
                with tc.tile_pool(name="xs0", bufs=4) as xp0, \
                     tc.tile_pool(name="pj0", bufs=1, space="PSUM") as pjps0:
                    xcur = {}

                    def xsrc0(k):
                        if k not in xcur:
                            nc.sync.dma_start(wt_tiles[k][:],
                                              wt[k * 128:(k + 1) * 128, :])
                            t = xp0.tile([128, 1024], BF16, tag="xs", name="xs")
                            nc.sync.dma_start(t[:], xh[k * 128:(k + 1) * 128, 0:1024])
                            xcur[k] = t
                        return xcur[k]
                    proj_group(list(range(8)), xsrc0, pjps0)
                    nc.sync.dma_start(convw_t[:], conv_w[:])
                    nc.sync.dma_start(convb_t[:], conv_b[:])
                    nc.sync.dma_start(ident[:], ident_d[:])
                    nc.sync.dma_start(beta_pos[:], bpos_d[:])
                    nc.sync.dma_start(beta_neg[:], bneg_d[:])

                with tc.tile_pool(name="cvps", bufs=3, space="PSUM") as cvps, \
                     tc.tile_pool(name="xs1", bufs=1) as xp1:
                    xs1 = {}

                    def xsrc1_load(k):
                        if k not in xs1:
                            t = xp1.tile([128, 1024], BF16, tag=f"x1{k}",
                                         name=f"x1{k}")
                            nc.sync.dma_start(t[:], xh[k * 128:(k + 1) * 128,
                                                       1024:2048])
                            xs1[k] = t
                        return xs1[k]

                    # group 1a: p=1, j=8..12 (5 banks) + conv nb=0
                    with tc.tile_pool(name="pj1", bufs=1, space="PSUM") as pjps1:
                        conv_emit(0, cvps)
                        proj_group([8, 9, 10, 11, 12], xsrc1_load, pjps1)
                        nc.sync.dma_start(mask_su[:], mask_su_d[:])
                        nc.sync.dma_start(mask_ui[:], mask_ui_d[:])
                        nc.sync.dma_start(outwt[:], outwt_d[:])

                    # group 1b: j=13..15 (3 banks) + conv nb=1,2,3
                    with tc.tile_pool(name="pj2", bufs=1, space="PSUM") as pjps2:
                        conv_emit(1, cvps)
                        proj_group([13, 14, 15], lambda k: xs1[k], pjps2)
                        conv_emit(2, cvps)
                        conv_emit(3, cvps)

                    # l2 scales
                    for s, rr in ((0, rk), (1, rq)):
                        ssq = bigpool.tile([128, 1], F32, tag=f"ssq{s}",
                                           name=f"ssq{s}")
                        nc.vector.tensor_reduce(ssq[:], ssq_acc[s][:],
                                                mybir.AxisListType.X,
                                                AluOpType.add)
                        sq = bigpool.tile([128, 1], F32, tag=f"sq{s}",
                                          name=f"sq{s}")
                        nc.scalar.activation(sq[:], ssq[:], ACT.Sqrt)
                        nc.vector.reciprocal(rr[:], sq[:])
                    nc.vector.tensor_mul(rk2[:], rk[:], rk[:])
                    nc.vector.tensor_mul(rkq[:], rk[:], rq[:])
                    if _DEBUG:
                        nc.sync.dma_start(dbg_y[:], Ybig[:])
                        for s3 in range(3):
                            nc.sync.dma_start(dbg_kqv[:, s3 * 2048:(s3 + 1) * 2048],
                                              kqv[s3][:])
                        nc.sync.dma_start(dbg_rkrq[:, 0:1], rk[:])
                        nc.sync.dma_start(dbg_rkrq[:, 1:2], rq[:])

            # ---- prepass + scan (interleaved, prepass leads by 2) ----
            with tc.tile_pool(name="pp1", bufs=1, space="PSUM") as pp1, \
                 tc.tile_pool(name="pvat", bufs=1, space="PSUM") as pvat, \
                 tc.tile_pool(name="ppc", bufs=2, space="PSUM") as ppc, \
                 tc.tile_pool(name="pscan", bufs=2, space="PSUM") as pscan, \
                 tc.tile_pool(name="psout", bufs=1, space="PSUM") as psout, \
                 tc.tile_pool(name="ppsb", bufs=3) as ppsb, \
                 tc.tile_pool(name="scansb", bufs=4) as scansb, \
                 tc.tile_pool(name="qnp", bufs=4) as qnp, \
                 tc.tile_pool(name="stp", bufs=3) as stp, \
                 tc.tile_pool(name="osb", bufs=2) as osb:

                pcs = [None] * NCHUNK
                sts = [None] * NCHUNK
                qns = [None] * NCHUNK
                spt = [None] * (NCHUNK + 1)

                def prepass(c):
                    cs = slice(c * 128, (c + 1) * 128)
                    bpos = beta_pos[:, c:c + 1]
                    bneg = beta_neg[:, c:c + 1]
                    # folded l2 operands for this chunk (Pool engine)
                    KK = ppsb.tile([128, 384], BF16, tag="kk", name="kk")
                    Ks2c, Kkqc, Knc = KK[:, 0:128], KK[:, 128:256], KK[:, 256:384]
                    nc.gpsimd.tensor_scalar_mul(Ks2c, kqv[0][:, cs], rk2[:])
                    nc.gpsimd.tensor_scalar_mul(Kkqc, kqv[0][:, cs], rkq[:])
                    nc.gpsimd.tensor_scalar_mul(Knc, kqv[0][:, cs], rk[:])
                    Qnc = qnp.tile([128, 128], BF16, tag="qn", name=f"qn{c}")
                    nc.gpsimd.tensor_scalar_mul(Qnc[:], kqv[1][:, cs], rq[:])
                    qns[c] = Qnc

                    p1 = pp1.tile([128, 512], F32, tag="p1", name="p1")
                    psG, psKQ = p1[:, 0:128], p1[:, 128:256]
                    psR, psF = p1[:, 256:384], p1[:, 384:512]
                    nc.tensor.matmul(psG, Ks2c, kqv[0][:, cs], start=True, stop=True)
                    nc.tensor.matmul(psKQ, Kkqc, kqv[1][:, cs], start=True, stop=True)
                    Nt = ppsb.tile([128, 128], BF16, tag="nt", name="nt")
                    nc.vector.tensor_scalar_mul(Nt[:], psG, bneg)
                    Pt = ppsb.tile([128, 128], BF16, tag="pt", name="pt")
                    nc.vector.tensor_mul(Pt[:], psKQ, mask_ui[:])
                    pva = pvat.tile([128, 384], BF16, tag="pva", name="pva")
                    psAt, psVt, psKtr = (pva[:, 0:128], pva[:, 128:256],
                                         pva[:, 256:384])
                    nc.tensor.transpose(psAt, Nt[:], ident[:])
                    nc.tensor.transpose(psVt, kqv[2][:, cs], ident[:])
                    nc.tensor.transpose(psKtr, Knc, ident[:])
                    At = ppsb.tile([128, 128], BF16, tag="at", name="at")
                    nc.vector.tensor_mul(At[:], psAt, mask_su[:])
                    VK = ppsb.tile([128, 256], BF16, tag="vk", name="vk")
                    Vb, KbN = VK[:, 0:128], VK[:, 128:256]
                    nc.vector.tensor_scalar_mul(Vb, psVt, bpos)
                    nc.vector.tensor_scalar_mul(KbN, psKtr, bneg)
                    Ktr = ppsb.tile([128, 128], BF16, tag="ktr", name="ktr")
                    nc.vector.tensor_copy(Ktr[:], psKtr)
                    # R = Vb + A Vb ; FN = KbN + A KbN
                    nc.tensor.matmul(psR, At[:], Vb, start=True, stop=False)
                    nc.tensor.matmul(psR, ident[:], Vb, start=False, stop=True)
                    nc.tensor.matmul(psF, At[:], KbN, start=True, stop=False)
                    nc.tensor.matmul(psF, ident[:], KbN, start=False, stop=True)
                    RF = ppsb.tile([128, 256], BF16, tag="rf", name="rf")
                    nc.scalar.activation(RF[:], p1[:, 256:512], ACT.Copy)
                    R, FN = RF[:, 0:128], RF[:, 128:256]
                    # ppc: J | HtN | PRT | GtN (all groups closed here)
                    pc = ppc.tile([128, 512], F32, tag="pc", name=f"pc{c}")
                    if c < NCHUNK - 1:
                        nc.tensor.matmul(pc[:, 0:128], Ktr[:], R,
                                         start=True, stop=True)
                        nc.tensor.matmul(pc[:, 128:256], FN, Ktr[:],
                                         start=True, stop=True)
                    nc.tensor.matmul(pc[:, 256:384], R, Pt[:],
                                     start=True, stop=True)
                    nc.tensor.matmul(pc[:, 384:512], FN, Pt[:], start=True, stop=True)
                    st = scansb.tile([128, 512], BF16, tag="st", name=f"st{c}")
                    if c < NCHUNK - 1:
                        nc.scalar.activation(st[:], pc[:], ACT.Copy)
                    else:
                        nc.scalar.activation(st[:, 256:512], pc[:, 256:512], ACT.Copy)
                    pcs[c], sts[c] = pc, st
                    if _DEBUG and c == 3:
                        nc.sync.dma_start(dbg_kk[:, 0:384], KK[:])
                        nc.sync.dma_start(dbg_kk[:, 384:512], Qnc[:])
                        nc.sync.dma_start(dbg_rf[:], RF[:])
                        nc.sync.dma_start(dbg_st[:], st[:])
                    if c == 0:
                        spt[1] = st[:, 0:128]   # S_1^T = J_0

                def scan(c):
                    st = sts[c]
                    J, HtN = st[:, 0:128], st[:, 128:256]
                    PRT, GtN = st[:, 256:384], st[:, 384:512]
                    ps = pscan.tile([128, 260], F32, tag="ps", name=f"ps{c}")
                    psS, psOT, psms = ps[:, 0:128], ps[:, 128:256], ps[:, 256:257]
                    if 1 <= c < NCHUNK - 1:
                        nc.tensor.matmul(psS, ident[:], spt[c], start=True, stop=False)
                        nc.tensor.matmul(psS, HtN, spt[c], start=False, stop=False)
                        nc.tensor.matmul(psS, ident[:], J, start=False, stop=True)
                    if c >= 1:
                        nc.tensor.matmul(psOT, spt[c], qns[c][:], start=True, stop=False)
                        nc.tensor.matmul(psOT, spt[c], GtN, start=False, stop=False)
                        nc.tensor.matmul(psOT, ident[:], PRT, start=False, stop=True)
                    if c == 0:
                        OT = PRT
                    elif c == NCHUNK - 1:
                        sot = scansb.tile([128, 256], BF16, tag="sot", name=f"sot{c}")
                        nc.scalar.activation(sot[:, 128:256], psOT, ACT.Copy)
                        OT = sot[:, 128:256]
                    else:
                        sot = scansb.tile([128, 256], BF16, tag="sot", name=f"sot{c}")
                        nc.scalar.activation(sot[:], ps[:, 0:256], ACT.Copy)
                        spt[c + 1] = sot[:, 0:128]
                        OT = sot[:, 128:256]
                        if _DEBUG and c == 3:
                            nc.sync.dma_start(dbg_spt[:], sot[:, 0:128])
                    Osq = scansb.tile([128, 128], BF16, tag="osq", name="osq")
                    nc.gpsimd.tensor_mul(Osq[:], OT, OT)
                    nc.tensor.matmul(psms, Osq[:], ones_c[:],
                                     start=True, stop=True, skip_group_check=True)
                    sqm = stp.tile([128, 1], F32, tag="sqm", name="sqm")
                    nc.scalar.activation(sqm[:], psms, ACT.Sqrt,
                                         bias=eps_c[:], scale=1.0 / 128.0)
                    rsm = stp.tile([128, 1], F32, tag="rsm", name="rsm")
                    nc.vector.reciprocal(rsm[:], sqm[:])
                    po = psout.tile([128, 1024], F32, tag="po", name="po")
                    for nb in range(2):
                        nc.tensor.matmul(po[:, nb * 512:(nb + 1) * 512], OT,
                                         outwt[:, nb * 512:(nb + 1) * 512],
                                         start=True, stop=True)
                    outsb = osb.tile([128, NOUT], F32, tag="outsb", name="outsb")
                    nc.vector.tensor_scalar_mul(outsb[:, 0:512], po[:, 0:512], rsm[:])
                    nc.sync.dma_start(out_sh[c * 128:(c + 1) * 128, 0:512],
                                      outsb[:, 0:512])
                    nc.scalar.activation(outsb[:, 512:1024], po[:, 512:1024],
                                         ACT.Copy, scale=rsm[:])
                    nc.sync.dma_start(out_sh[c * 128:(c + 1) * 128, 512:1024],
                                      outsb[:, 512:1024])

                prepass(0)
                prepass(1)
                for c in range(NCHUNK):
                    if c + 2 < NCHUNK:
                        prepass(c + 2)
                    scan(c)

    nc.compile()
    return nc


_prog_cache = {}
_TRACE = False
_LAST_RES = None


def kernel(**inputs):
    from concourse import mybir
    from concourse.bass_utils import run_bass_kernel_spmd

    np32 = np.float32
    bf16 = mybir.dt.np(mybir.dt.bfloat16)

    x = np.asarray(inputs["x"], np32)
    beta_b = float(np.asarray(inputs["beta_b"]).reshape(-1)[0])
    eps_rms = float(np.finfo(np.float32).eps)

    key = (eps_rms,)
    if key not in _prog_cache:
        _prog_cache[key] = _build_program(eps_rms)
    nc = _prog_cache[key]

    i = np.arange(L)
    perm = 16 * (i % 128) + (i // 128)
    wt = np.concatenate([np.asarray(inputs["k_proj_w"], np32).T,
                         np.asarray(inputs["q_proj_w"], np32).T,
                         np.asarray(inputs["v_proj_w"], np32).T,
                         np.zeros((L, 2), np32)], axis=1)
    bias_row = np.concatenate(
        [np.asarray(inputs["k_proj_b"], np32),
         np.asarray(inputs["q_proj_b"], np32),
         np.asarray(inputs["v_proj_b"], np32),
         np.zeros(2, np32)])[None, :]
    conv_w = np.zeros((128, 1152), np32)
    for s, name in enumerate(["k_conv_w", "q_conv_w", "v_conv_w"]):
        w = np.asarray(inputs[name], np32)
        for t in range(3):
            conv_w[:, (3 * s + t) * 128:(3 * s + t + 1) * 128] = w[:, :, t, 1].T
    conv_b = np.stack([np.asarray(inputs["k_conv_b"], np32),
                       np.asarray(inputs["q_conv_b"], np32),
                       np.asarray(inputs["v_conv_b"], np32)], axis=1)
    ident = np.eye(128, dtype=np32)
    r = np.arange(128)
    mask_su = (r[:, None] < r[None, :]).astype(np32)
    mask_ui = (r[:, None] <= r[None, :]).astype(np32)
    outw_eff = (np.asarray(inputs["out_w"], np32) *
                np.asarray(inputs["rms_w"], np32)[None, :]).T  # (128, 2048)
    out_b = np.asarray(inputs["out_b"], np32)

    beta_w = np.asarray(inputs["beta_w"], np32).reshape(-1)
    beta_all = 1.0 / (1.0 + np.exp(-(x.reshape(-1, L) @ beta_w + beta_b)))
    beta_all = beta_all.reshape(B, NCHUNK, 128)

    in_maps = []
    for core in range(8):
        b, h = core // 2, core % 2
        xcore = np.ascontiguousarray(x[b][perm, :].T).astype(bf16)
        bpos = np.ascontiguousarray(beta_all[b].T).astype(np32)
        in_maps.append({
            "xh": xcore,
            "wt": wt.astype(bf16),
            "bias_row": bias_row.astype(bf16),
            "conv_w": conv_w.astype(bf16),
            "conv_b": conv_b,
            "ident": ident.astype(bf16),
            "mask_su": mask_su.astype(bf16),
            "mask_ui": mask_ui.astype(bf16),
            "outwt": np.ascontiguousarray(
                outw_eff[:, h * NOUT:(h + 1) * NOUT]).astype(bf16),
            "bpos": bpos,
            "bneg": -bpos,
        })

    res = run_bass_kernel_spmd(nc, in_maps, core_ids=list(range(8)),
                               trace=_TRACE)
    global _LAST_RES
    _LAST_RES = res
    if _TRACE and res.exec_time_ns is not None:
        print("HW exec time: %d ns" % res.exec_time_ns)
    out = np.empty((B, L, L), np32)
    for b in range(B):
        out[b, :, :NOUT] = res.results[2 * b]["out_sh"] + out_b[None, :NOUT]
        out[b, :, NOUT:] = res.results[2 * b + 1]["out_sh"] + out_b[None, NOUT:]
    return out


# revision 35
# speedup vs baseline: 1.0804x; 1.0804x over previous
"""DeltaNetBlock Trainium2 kernel (v4).

Sharding: 8 cores = 4 batches x 2 output-column halves. Each core computes
the full middle (proj -> conv -> silu -> l2norm -> chunked delta scan) for
its batch and the output projection for its half of the output columns.

Structure:
- Projection in 3 groups (8m / 5m / 3m PSUM banks); conv tiles interleave
  with the later groups so silu/l2 finish right after the projection.
- beta = sigmoid(x @ beta_w + b) computed on the host (input prep).
- l2 normalization folded into matmul operands per chunk on the Pool
  engine: Ks2 = rk^2*k', Kkq = rk*rq*k', Qn = rq*q', Kn = rk*k'.
- Per 128-chunk the delta recurrence is affine in S^T:
    S_{c+1}^T = S_c^T - H_c S_c^T + J_c,  O_c^T = S Qn_c - (P F S^T)^T + (P R)^T
  with R = (I+A) beta V^T, FN = -(I+A) beta Kn^T, J = Kn R, HtN = -H^T,
  GtN = -(P F)^T, PRT = (P R)^T all S-independent. J and PRT never leave
  PSUM: the scan's S/O matmuls continue their accumulation groups in place.
- rmsnorm via ones-matmul column sums; rsqrt folded into the out-proj
  result copies (per-partition scale). out_b added on host.
"""
import sys
sys.path.insert(0, '/opt/trn_rl_repo')
import numpy as np

B, L, D = 4, 2048, 128
NCHUNK = L // 128
NOUT = L // 2
HORNER = 1
_DEBUG = False


def _build_program(eps_rms: float):
    from concourse import bacc, mybir, tile

    F32 = mybir.dt.float32
    BF16 = mybir.dt.bfloat16
    ACT = mybir.ActivationFunctionType
    from concourse.alu_op_type import AluOpType

    nc = bacc.Bacc("TRN2", target_bir_lowering=False, debug=False)

    xh = nc.dram_tensor("xh", [L, L], BF16, kind="ExternalInput")
    wt = nc.dram_tensor("wt", [L, 386], BF16, kind="ExternalInput")
    bias_row_d = nc.dram_tensor("bias_row", [1, 386], BF16, kind="ExternalInput")
    conv_w = nc.dram_tensor("conv_w", [128, 1152], BF16, kind="ExternalInput")
    conv_b = nc.dram_tensor("conv_b", [128, 3], F32, kind="ExternalInput")
    ident_d = nc.dram_tensor("ident", [128, 128], BF16, kind="ExternalInput")
    mask_su_d = nc.dram_tensor("mask_su", [128, 128], BF16, kind="ExternalInput")
    mask_ui_d = nc.dram_tensor("mask_ui", [128, 128], BF16, kind="ExternalInput")
    outwt_d = nc.dram_tensor("outwt", [128, NOUT], BF16, kind="ExternalInput")
    bpos_d = nc.dram_tensor("bpos", [128, 16], F32, kind="ExternalInput")
    bneg_d = nc.dram_tensor("bneg", [128, 16], F32, kind="ExternalInput")
    out_sh = nc.dram_tensor("out_sh", [L, NOUT], F32, kind="ExternalOutput")
    if _DEBUG:
        dbg_y = nc.dram_tensor("dbg_y", [128, 6150], BF16, kind="ExternalOutput")
        dbg_kqv = nc.dram_tensor("dbg_kqv", [128, 3 * 2048], BF16, kind="ExternalOutput")
        dbg_rkrq = nc.dram_tensor("dbg_rkrq", [128, 2], F32, kind="ExternalOutput")
        dbg_st = nc.dram_tensor("dbg_st", [128, 512], BF16, kind="ExternalOutput")
        dbg_spt = nc.dram_tensor("dbg_spt", [128, 128], BF16, kind="ExternalOutput")
        dbg_kk = nc.dram_tensor("dbg_kk", [128, 512], BF16, kind="ExternalOutput")
        dbg_rf = nc.dram_tensor("dbg_rf", [128, 256], BF16, kind="ExternalOutput")

    with tile.TileContext(nc) as tc:
        with tc.tile_pool(name="const", bufs=1) as cpool, \
             tc.tile_pool(name="big", bufs=1) as bigpool, \
             tc.tile_pool(name="vbp", bufs=1) as vbp:

            ones_r = cpool.tile([1, 128], BF16)
            nc.vector.memset(ones_r[:], 1.0)
            ones_c = cpool.tile([128, 1], BF16)
            nc.vector.memset(ones_c[:], 1.0)
            eps_c = cpool.tile([128, 1], F32)
            nc.vector.memset(eps_c[:], float(eps_rms))

            bias_row = cpool.tile([1, 386], BF16)
            convw_t = cpool.tile([128, 1152], BF16)
            convb_t = cpool.tile([128, 3], F32)
            ident = cpool.tile([128, 128], BF16)
            mask_su = cpool.tile([128, 128], BF16)
            mask_ui = cpool.tile([128, 128], BF16)
            outwt = cpool.tile([128, NOUT], BF16)
            beta_pos = cpool.tile([128, 16], F32)
            beta_neg = cpool.tile([128, 16], F32)
            rk = cpool.tile([128, 1], F32)
            rq = cpool.tile([128, 1], F32)
            rk2 = cpool.tile([128, 1], F32)
            rkq = cpool.tile([128, 1], F32)

            Ybig = bigpool.tile([128, 6150], BF16, name="ybig")
            Y = [Ybig[:, s * 2050:(s + 1) * 2050] for s in range(3)]
            kqv = [bigpool.tile([128, 2048], BF16, tag=f"c{s}", name=f"c{s}")
                   for s in range(3)]

            nc.sync.dma_start(bias_row[:], bias_row_d[:])
            for s in range(3):
                nc.vector.memset(Y[s][:, 0:1], 0.0)
                nc.vector.memset(Y[s][:, 2049:2050], 0.0)

            ssq_acc = {s: cpool.tile([128, 4], F32, name=f"acc{s}")
                       for s in (0, 1)}
            sq_scr = {s: bigpool.tile([128, 512], BF16, tag=f"scr{s}",
                                      name=f"scr{s}") for s in (0, 1)}

            def conv_emit(nb, cvps):
                for s in (0, 1, 2):
                    ps = cvps.tile([128, 512], F32, tag="cv", name="cv")
                    for t in range(3):
                        nc.tensor.matmul(
                            ps[:],
                            convw_t[:, (3 * s + t) * 128:(3 * s + t + 1) * 128],
                            Y[s][:, nb * 512 + t:nb * 512 + t + 512],
                            start=(t == 0), stop=(t == 2))
                    kt = kqv[s][:, nb * 512:(nb + 1) * 512]
                    nc.scalar.activation(kt, ps[:], ACT.Silu,
                                         bias=convb_t[:, s:s + 1], scale=1.0)
                    if s in (0, 1):
                        nc.vector.scalar_tensor_tensor(
                            sq_scr[s][:], kt, 1.0, kt,
                            AluOpType.bypass, AluOpType.mult,
                            accum_out=ssq_acc[s][:, nb:nb + 1])

            def proj_group(jlist, xsrc, pjps):
                pj = [pjps.tile([128, 386], F32, tag=f"pj{i}", name=f"pj{i}")
                      for i in range(len(jlist))]
                for k in range(16):
                    xt = xsrc(k)
                    for i, j in enumerate(jlist):
                        nc.tensor.matmul(
                            pj[i][:], xt[:, (j % 8) * 128:(j % 8 + 1) * 128],
                            wt_tiles[k][:], start=(k == 0), stop=False)
                for i, j in enumerate(jlist):
                    nc.tensor.matmul(pj[i][:], ones_r[:], bias_row[:],
                                     start=False, stop=True)
                    nc.scalar.activation(
                        Ybig[:].rearrange("p (s r) -> p s r", s=3)[
                            :, :, 1 + 128 * j:129 + 128 * j],
                        pj[i][:, 0:384].rearrange("p (s c) -> p s c", s=3),
                        ACT.Copy)

            vbs = [None] * NCHUNK
            with tc.tile_pool(name="wtp", bufs=1) as wtpool:
                wt_tiles = [wtpool.tile([128, 386], BF16, tag=f"wt{k}",
                                        name=f"wt{k}") for k in range(16)]

                # group 0: p=0, all 8 j-blocks (8 PSUM banks), DMA interleaved
                with tc.tile_pool(name="xs0", bufs=4) as xp0, \
                     tc.tile_pool(name="pj0", bufs=1, space="PSUM") as pjps0:
                    xcur = {}

                    def xsrc0(k):
                        if k not in xcur:
                            nc.sync.dma_start(wt_tiles[k][:],
                                              wt[k * 128:(k + 1) * 128, :])
                            t = xp0.tile([128, 1024], BF16, tag="xs", name="xs")
                            nc.sync.dma_start(t[:], xh[k * 128:(k + 1) * 128, 0:1024])
                            xcur[k] = t
                        return xcur[k]
                    proj_group(list(range(8)), xsrc0, pjps0)
                    nc.sync.dma_start(convw_t[:], conv_w[:])
                    nc.sync.dma_start(convb_t[:], conv_b[:])
                    nc.sync.dma_start(ident[:], ident_d[:])
                    nc.sync.dma_start(beta_pos[:], bpos_d[:])
                    nc.sync.dma_start(beta_neg[:], bneg_d[:])

                with tc.tile_pool(name="cvps", bufs=3, space="PSUM") as cvps, \
                     tc.tile_pool(name="xs1", bufs=1) as xp1:
                    xs1 = {}

                    def xsrc1_load(k):
                        if k not in xs1:
                            t = xp1.tile([128, 1024], BF16, tag=f"x1{k}",
                                         name=f"x1{k}")
                            nc.sync.dma_start(t[:], xh[k * 128:(k + 1) * 128,
                                                       1024:2048])
                            xs1[k] = t
                        return xs1[k]

                    # group 1a: p=1, j=8..12 (5 banks) + conv nb=0
                    with tc.tile_pool(name="pj1", bufs=1, space="PSUM") as pjps1:
                        conv_emit(0, cvps)
                        proj_group([8, 9, 10, 11, 12], xsrc1_load, pjps1)
                        nc.sync.dma_start(mask_su[:], mask_su_d[:])
                        nc.sync.dma_start(mask_ui[:], mask_ui_d[:])
                        nc.sync.dma_start(outwt[:], outwt_d[:])

                    with tc.tile_pool(name="pvps", bufs=2, space="PSUM") as pvps:
                        def vb_pre(c):
                            cs = slice(c * 128, (c + 1) * 128)
                            psv = pvps.tile([128, 128], BF16, tag="psv",
                                            name=f"psv{c}")
                            nc.tensor.transpose(psv[:], kqv[2][:, cs], ident[:])
                            vt = vbp.tile([128, 128], BF16, tag=f"vb{c}",
                                          name=f"vb{c}")
                            nc.vector.tensor_scalar_mul(
                                vt[:], psv[:], beta_pos[:, c:c + 1])
                            vbs[c] = vt

                        # group 1b: j=13..15 (3 banks) + conv nb=1,2,3
                        with tc.tile_pool(name="pj2", bufs=1, space="PSUM") as pjps2:
                            conv_emit(1, cvps)
                            proj_group([13, 14, 15], lambda k: xs1[k], pjps2)
                        for c in range(8):
                            vb_pre(c)
                        conv_emit(2, cvps)
                        conv_emit(3, cvps)
                        for c in range(8, 16):
                            vb_pre(c)

                    # l2 scales
                    for s, rr in ((0, rk), (1, rq)):
                        ssq = bigpool.tile([128, 1], F32, tag=f"ssq{s}",
                                           name=f"ssq{s}")
                        nc.vector.tensor_reduce(ssq[:], ssq_acc[s][:],
                                                mybir.AxisListType.X,
                                                AluOpType.add)
                        sq = bigpool.tile([128, 1], F32, tag=f"sq{s}",
                                          name=f"sq{s}")
                        nc.scalar.activation(sq[:], ssq[:], ACT.Sqrt)
                        nc.vector.reciprocal(rr[:], sq[:])
                    nc.vector.tensor_mul(rk2[:], rk[:], rk[:])
                    nc.vector.tensor_mul(rkq[:], rk[:], rq[:])
                    if _DEBUG:
                        nc.sync.dma_start(dbg_y[:], Ybig[:])
                        for s3 in range(3):
                            nc.sync.dma_start(dbg_kqv[:, s3 * 2048:(s3 + 1) * 2048],
                                              kqv[s3][:])
                        nc.sync.dma_start(dbg_rkrq[:, 0:1], rk[:])
                        nc.sync.dma_start(dbg_rkrq[:, 1:2], rq[:])

            # ---- prepass + scan (interleaved, prepass leads by 2) ----
            with tc.tile_pool(name="pp1", bufs=2, space="PSUM") as pp1, \
                 tc.tile_pool(name="pvat", bufs=2, space="PSUM") as pvat, \
                 tc.tile_pool(name="ppc", bufs=1, space="PSUM") as ppc, \
                 tc.tile_pool(name="pscan", bufs=1, space="PSUM") as pscan, \
                 tc.tile_pool(name="psout", bufs=1, space="PSUM") as psout, \
                 tc.tile_pool(name="ppsb", bufs=6) as ppsb, \
                 tc.tile_pool(name="scansb", bufs=6) as scansb, \
                 tc.tile_pool(name="qnp", bufs=6) as qnp, \
                 tc.tile_pool(name="stp", bufs=4) as stp, \
                 tc.tile_pool(name="osb", bufs=3) as osb:

                pcs = [None] * NCHUNK
                sts = [None] * NCHUNK
                qns = [None] * NCHUNK
                spt = [None] * (NCHUNK + 1)

                def prepass(c):
                    cs = slice(c * 128, (c + 1) * 128)
                    bpos = beta_pos[:, c:c + 1]
                    bneg = beta_neg[:, c:c + 1]
                    # folded l2 operands for this chunk (Pool engine)
                    KK = ppsb.tile([128, 384], BF16, tag="kk", name="kk")
                    Ks2c, Kkqc, Knc = KK[:, 0:128], KK[:, 128:256], KK[:, 256:384]
                    nc.gpsimd.tensor_scalar_mul(Ks2c, kqv[0][:, cs], rk2[:])
                    nc.gpsimd.tensor_scalar_mul(Kkqc, kqv[0][:, cs], rkq[:])
                    nc.gpsimd.tensor_scalar_mul(Knc, kqv[0][:, cs], rk[:])
                    Qnc = qnp.tile([128, 128], BF16, tag="qn", name=f"qn{c}")
                    nc.gpsimd.tensor_scalar_mul(Qnc[:], kqv[1][:, cs], rq[:])
                    qns[c] = Qnc

                    p1 = pp1.tile([128, 512], F32, tag="p1", name="p1")
                    psG, psKQ = p1[:, 0:128], p1[:, 128:256]
                    psR, psF = p1[:, 256:384], p1[:, 384:512]
                    nc.tensor.matmul(psG, Ks2c, kqv[0][:, cs], start=True, stop=True)
                    nc.tensor.matmul(psKQ, Kkqc, kqv[1][:, cs], start=True, stop=True)
                    Nt = ppsb.tile([128, 128], BF16, tag="nt", name="nt")
                    nc.vector.tensor_scalar_mul(Nt[:], psG, bneg)
                    Pt = ppsb.tile([128, 128], BF16, tag="pt", name="pt")
                    nc.vector.tensor_mul(Pt[:], psKQ, mask_ui[:])
                    pva = pvat.tile([128, 256], BF16, tag="pva", name="pva")
                    psAt, psKtr = pva[:, 0:128], pva[:, 128:256]
                    nc.tensor.transpose(psAt, Nt[:], ident[:])
                    nc.tensor.transpose(psKtr, Knc, ident[:])
                    At = ppsb.tile([128, 128], BF16, tag="at", name="at")
                    nc.vector.tensor_mul(At[:], psAt, mask_su[:])
                    Vb = vbs[c][:]
                    KbN = ppsb.tile([128, 128], BF16, tag="kbn", name="kbn")
                    nc.vector.tensor_scalar_mul(KbN[:], psKtr, bneg)
                    Ktr = ppsb.tile([128, 128], BF16, tag="ktr", name="ktr")
                    nc.vector.tensor_copy(Ktr[:], psKtr)
                    # R = Vb + A Vb ; FN = KbN + A KbN
                    nc.tensor.matmul(psR, At[:], Vb, start=True, stop=False)
                    nc.tensor.matmul(psR, ident[:], Vb, start=False, stop=True)
                    nc.tensor.matmul(psF, At[:], KbN[:], start=True, stop=False)
                    nc.tensor.matmul(psF, ident[:], KbN[:], start=False, stop=True)
                    RF = ppsb.tile([128, 256], BF16, tag="rf", name="rf")
                    nc.scalar.activation(RF[:], p1[:, 256:512], ACT.Copy)
                    R, FN = RF[:, 0:128], RF[:, 128:256]
                    # ppc: J | HtN | PRT | GtN (all groups closed here)
                    pc = ppc.tile([128, 512], F32, tag="pc", name=f"pc{c}")
                    if c < NCHUNK - 1:
                        nc.tensor.matmul(pc[:, 0:128], Ktr[:], R,
                                         start=True, stop=True)
                        nc.tensor.matmul(pc[:, 128:256], FN, Ktr[:],
                                         start=True, stop=True)
                    nc.tensor.matmul(pc[:, 256:384], R, Pt[:],
                                     start=True, stop=True)
                    nc.tensor.matmul(pc[:, 384:512], FN, Pt[:], start=True, stop=True)
                    st = scansb.tile([128, 512], BF16, tag="st", name=f"st{c}")
                    if c < NCHUNK - 1:
                        nc.scalar.activation(st[:], pc[:], ACT.Copy)
                    else:
                        nc.scalar.activation(st[:, 256:512], pc[:, 256:512], ACT.Copy)
                    pcs[c], sts[c] = pc, st
                    if _DEBUG and c == 3:
                        nc.sync.dma_start(dbg_kk[:, 0:384], KK[:])
                        nc.sync.dma_start(dbg_kk[:, 384:512], Qnc[:])
                        nc.sync.dma_start(dbg_rf[:], RF[:])
                        nc.sync.dma_start(dbg_st[:], st[:])
                    if c == 0:
                        spt[1] = st[:, 0:128]   # S_1^T = J_0

                def scan(c):
                    st = sts[c]
                    J, HtN = st[:, 0:128], st[:, 128:256]
                    PRT, GtN = st[:, 256:384], st[:, 384:512]
                    ps = pscan.tile([128, 260], F32, tag="ps", name=f"ps{c}")
                    psS, psOT, psms = ps[:, 0:128], ps[:, 128:256], ps[:, 256:257]
                    if 1 <= c < NCHUNK - 1:
                        nc.tensor.matmul(psS, ident[:], spt[c], start=True, stop=False)
                        nc.tensor.matmul(psS, HtN, spt[c], start=False, stop=False)
                        nc.tensor.matmul(psS, ident[:], J, start=False, stop=True)
                    if c >= 1:
                        nc.tensor.matmul(psOT, spt[c], qns[c][:], start=True, stop=False)
                        nc.tensor.matmul(psOT, spt[c], GtN, start=False, stop=False)
                        nc.tensor.matmul(psOT, ident[:], PRT, start=False, stop=True)
                    if c == 0:
                        OT = PRT
                    elif c == NCHUNK - 1:
                        sot = scansb.tile([128, 256], BF16, tag="sot", name=f"sot{c}")
                        nc.scalar.activation(sot[:, 128:256], psOT, ACT.Copy)
                        OT = sot[:, 128:256]
                    else:
                        sot = scansb.tile([128, 256], BF16, tag="sot", name=f"sot{c}")
                        nc.scalar.activation(sot[:], ps[:, 0:256], ACT.Copy)
                        spt[c + 1] = sot[:, 0:128]
                        OT = sot[:, 128:256]
                        if _DEBUG and c == 3:
                            nc.sync.dma_start(dbg_spt[:], sot[:, 0:128])
                    Osq = scansb.tile([128, 128], BF16, tag="osq", name="osq")
                    nc.gpsimd.tensor_mul(Osq[:], OT, OT)
                    nc.tensor.matmul(psms, Osq[:], ones_c[:],
                                     start=True, stop=True, skip_group_check=True)
                    sqm = stp.tile([128, 1], F32, tag="sqm", name="sqm")
                    nc.scalar.activation(sqm[:], psms, ACT.Sqrt,
                                         bias=eps_c[:], scale=1.0 / 128.0)
                    rsm = stp.tile([128, 1], F32, tag="rsm", name="rsm")
                    nc.vector.reciprocal(rsm[:], sqm[:])
                    po = psout.tile([128, 1024], F32, tag="po", name="po")
                    for nb in range(2):
                        nc.tensor.matmul(po[:, nb * 512:(nb + 1) * 512], OT,
                                         outwt[:, nb * 512:(nb + 1) * 512],
                                         start=True, stop=True)
                    outsb = osb.tile([128, NOUT], F32, tag="outsb", name="outsb")
                    nc.vector.tensor_scalar_mul(outsb[:, 0:512], po[:, 0:512], rsm[:])
                    nc.sync.dma_start(out_sh[c * 128:(c + 1) * 128, 0:512],
                                      outsb[:, 0:512])
                    nc.scalar.activation(outsb[:, 512:1024], po[:, 512:1024],
                                         ACT.Copy, scale=rsm[:])
                    nc.sync.dma_start(out_sh[c * 128:(c + 1) * 128, 512:1024],
                                      outsb[:, 512:1024])

                LEAD = 4
                for c in range(min(LEAD, NCHUNK)):
                    prepass(c)
                for c in range(NCHUNK):
                    scan(c)
                    if c + LEAD < NCHUNK:
                        prepass(c + LEAD)

    nc.compile()
    return nc


_prog_cache = {}
_TRACE = False
_LAST_RES = None


def kernel(**inputs):
    from concourse import mybir
    from concourse.bass_utils import run_bass_kernel_spmd

    np32 = np.float32
    bf16 = mybir.dt.np(mybir.dt.bfloat16)

    x = np.asarray(inputs["x"], np32)
    beta_b = float(np.asarray(inputs["beta_b"]).reshape(-1)[0])
    eps_rms = float(np.finfo(np.float32).eps)

    key = (eps_rms,)
    if key not in _prog_cache:
        _prog_cache[key] = _build_program(eps_rms)
    nc = _prog_cache[key]

    i = np.arange(L)
    perm = 16 * (i % 128) + (i // 128)
    wt = np.concatenate([np.asarray(inputs["k_proj_w"], np32).T,
                         np.asarray(inputs["q_proj_w"], np32).T,
                         np.asarray(inputs["v_proj_w"], np32).T,
                         np.zeros((L, 2), np32)], axis=1)
    bias_row = np.concatenate(
        [np.asarray(inputs["k_proj_b"], np32),
         np.asarray(inputs["q_proj_b"], np32),
         np.asarray(inputs["v_proj_b"], np32),
         np.zeros(2, np32)])[None, :]
    conv_w = np.zeros((128, 1152), np32)
    for s, name in enumerate(["k_conv_w", "q_conv_w", "v_conv_w"]):
        w = np.asarray(inputs[name], np32)
        for t in range(3):
            conv_w[:, (3 * s + t) * 128:(3 * s + t + 1) * 128] = w[:, :, t, 1].T
    conv_b = np.stack([np.asarray(inputs["k_conv_b"], np32),
                       np.asarray(inputs["q_conv_b"], np32),
                       np.asarray(inputs["v_conv_b"], np32)], axis=1)
    ident = np.eye(128, dtype=np32)
    r = np.arange(128)
    mask_su = (r[:, None] < r[None, :]).astype(np32)
    mask_ui = (r[:, None] <= r[None, :]).astype(np32)
    outw_eff = (np.asarray(inputs["out_w"], np32) *
                np.asarray(inputs["rms_w"], np32)[None, :]).T  # (128, 2048)
    out_b = np.asarray(inputs["out_b"], np32)

    beta_w = np.asarray(inputs["beta_w"], np32).reshape(-1)
    beta_all = 1.0 / (1.0 + np.exp(-(x.reshape(-1, L) @ beta_w + beta_b)))
    beta_all = beta_all.reshape(B, NCHUNK, 128)

    in_maps = []
    for core in range(8):
        b, h = core // 2, core % 2
        xcore = np.ascontiguousarray(x[b][perm, :].T).astype(bf16)
        bpos = np.ascontiguousarray(beta_all[b].T).astype(np32)
        in_maps.append({
            "xh": xcore,
            "wt": wt.astype(bf16),
            "bias_row": bias_row.astype(bf16),
            "conv_w": conv_w.astype(bf16),
            "conv_b": conv_b,
            "ident": ident.astype(bf16),
            "mask_su": mask_su.astype(bf16),
            "mask_ui": mask_ui.astype(bf16),
            "outwt": np.ascontiguousarray(
                outw_eff[:, h * NOUT:(h + 1) * NOUT]).astype(bf16),
            "bpos": bpos,
            "bneg": -bpos,
        })

    res = run_bass_kernel_spmd(nc, in_maps, core_ids=list(range(8)),
                               trace=_TRACE)
    global _LAST_RES
    _LAST_RES = res
    if _TRACE and res.exec_time_ns is not None:
        print("HW exec time: %d ns" % res.exec_time_ns)
    out = np.empty((B, L, L), np32)
    for b in range(B):
        out[b, :, :NOUT] = res.results[2 * b]["out_sh"] + out_b[None, :NOUT]
        out[b, :, NOUT:] = res.results[2 * b + 1]["out_sh"] + out_b[None, NOUT:]
    return out
